# revision 1
# baseline (speedup 1.0000x reference)
"""Trainium2 Bass kernel for the LeNet C3 dense-conv layer.

Computes out = conv2d_valid(x, K, stride 1) + bias where K is the dense
[16, 6, 5, 5] kernel scattered from the sparse per-branch weights
(w3/w4/w6), x is [128, 6, 256, 256] f32, out is [128, 16, 252, 252] f32.

Strategy (v2):
  - Pure data parallelism: 16 images per NeuronCore across 8 cores.
  - Conv as shift-accumulated banded matmuls into PSUM, as in v1, but
    with COLUMN-GROUP TILED matmuls: instead of one M=96 matmul per
    image-pair x kx-pair, issue four concurrent M=32 matmuls (one per
    32-column PE array group, tile_position=(0,32c)) covering four
    (image-pair, r-pair) tasks at once into the four 32-partition
    slices of one PSUM bank.  Measured on HW: per-pipe issue cadence
    ~259 ns for N=512 with 4 pipes overlapped -> ~742 ns per 12-MM
    round vs ~3x287 ns for the M=96 serial form (1.55x tensor-engine
    speedup; LDWEIGHTS is not hidden in the serial form).
  - K=120 = two stacked copies of the 10 input rows (60 partitions
    each), second copy pre-shifted one column, so each matmul covers
    two kernel columns kx: 3 matmuls per task (kx {0,1} {2,3} {4}).
    K=120 > 96 keeps all four PE row-group quarters streaming at full
    rate (K<=96 measures at half rate on HW).
  - Input is staged to DRAM ONCE (60 rows, no duplication) and the
    shifted second copy is built ON-CHIP by a GpSimd copy (SBUF->SBUF,
    does not touch HBM/DMA).  This halves input HBM traffic vs v1:
    total DMA drops 73.9 MB -> 53.2 MB per core (DMA was 96% busy).
  - Two row-blocks (12 output rows) share one input tile / one input
    DMA (~1 MB) and one output DMA (~1.5 MB) for DMA efficiency.
  - PSUM bank [128, 512] per round; evictions alternate between the
    vector and scalar engines (different banks -> legal concurrent
    PSUM reads), bias add fused, fp16 cast.
  - fp16 operands (~3e-4 rel err; accumulation is fp32 in PSUM).
"""

import numpy as np

# LeNet-5 C3 sparse channel connectivity (from the model definition).
CH3 = np.array([[0, 1, 2], [1, 2, 3], [2, 3, 4], [3, 4, 5], [0, 4, 5], [0, 1, 5]])
CH4 = np.array([[0, 1, 2, 3], [1, 2, 3, 4], [2, 3, 4, 5], [0, 3, 4, 5],
                [0, 1, 4, 5], [0, 1, 2, 5], [0, 1, 3, 4], [1, 2, 4, 5],
                [0, 2, 3, 5]])

B, C, H, W = 128, 6, 256, 256
CO, HO, WO = 16, 252, 252
NCORES = 8
BPC = B // NCORES           # images per core (16)
KH = KW = 5

R = 6                       # output rows per block
HI = R + 4                  # input rows per block (10)
NBLK = HO // R              # 42 blocks
NSUP = NBLK // 2            # 21 superblocks (2 blocks each)
KK = C * HI                 # contraction rows per kx copy (60)
KP = 64                     # copy-0 rows padded to 64 (32-aligned engine APs)
TW = 4 + BPC * W + 1        # input tile width per block (4101, last col zero)
NRND = 6                    # PSUM rounds per block (4 tasks each)

_STATE = None  # cached Bass module so repeat kernel() calls skip re-tracing


def _dense_kernel(w3, w4, w6):
    k = np.zeros((CO, C, KH, KW), np.float32)
    k[np.arange(6)[:, None], CH3] = w3
    k[6 + np.arange(9)[:, None], CH4] = w4
    k[15] = w6[0]
    return k


# Tile partition layout: quarters [0:32]=copy0 rows 0..31,
# [32:64]=copy1 rows 0..31, [64:96]=copy0 rows 32..63,
# [96:128]=copy1 rows 32..63.  Input DMA writes quarters 0 and 2
# (partition ranges 0..31 and 64..95 -> all 16 SDMA engines; a plain
# [0:64] write would hit only the 8 even engines).  The dup engine
# copies build quarters 1 and 3.  Row index = i*6 + ci (>=KK is pad).


def _wall(kd, d, rp):
    """Banded lhsT [2*KP, 32] for kx offset d and r-pair rp, with the
    quartered partition layout above.  Column co*2 + rloc, value
    kd[co, ci, i - (2*rp + rloc), d + s]."""
    out = np.zeros((2 * KP, 32), np.float32)
    for p in range(2 * KP):
        q, r = divmod(p, 32)
        s = q % 2                   # copy (kx shift)
        row = 32 * (q // 2) + r     # i*6 + ci
        if row >= KK:
            continue
        i, ci = divmod(row, C)
        kx = d + s
        if kx >= KW:
            continue
        for rloc in range(2):
            ky = i - (2 * rp + rloc)
            if 0 <= ky < KH:
                out[p, np.arange(CO) * 2 + rloc] = kd[:, ci, ky, kx]
    return out


def _build_module():
    import concourse.bacc as bacc
    import concourse.mybir as mybir
    from concourse.tile import TileContext

    f32 = mybir.dt.float32
    f16 = mybir.dt.float16

    nc = bacc.Bacc(None)
    # Single-copy per-block input rows, flat: x[r, g*TW + c] = block g
    # row r col c.  Supertiles of up to 4 blocks share one input DMA.
    x_d = nc.dram_tensor("x", [KP, NBLK * TW], f16, kind="ExternalInput")
    # walls: [(d, rpair) -> [120, 32]] flattened to [120, 9*32]
    wall_d = nc.dram_tensor("wall", [2 * KP, 9 * 32], f16, kind="ExternalInput")
    b1_d = nc.dram_tensor("b1", [128, 1], f32, kind="ExternalInput")
    # o[sup, p, half*NRND+rnd, j*256+w'] fp16 (full PSUM banks incl. halo
    # cols; host slices w' 4:256).  Host unpacks (see kernel()).
    o_d = nc.dram_tensor("o", [NSUP, 128, 2 * NRND, 512], f16,
                         kind="ExternalOutput")

    with TileContext(nc) as tc:
        with (
            tc.tile_pool(name="wpool", bufs=1) as wp,
            tc.tile_pool(name="inpool", bufs=4) as ip,
            tc.tile_pool(name="outpool", bufs=3) as op,
            tc.tile_pool(name="pspool", bufs=4, space="PSUM") as pp,
        ):
            wall_t = wp.tile([2 * KP, 9 * 32], f16)
            nc.sync.dma_start(wall_t[:], wall_d[:])
            b1_t = wp.tile([128, 1], f32)
            nc.sync.dma_start(b1_t[:], b1_d[:])

            # Prime the engines / constant tiles so steady-state
            # instructions carry few semaphore waits.
            prime_ps = pp.tile([128, 2, 512], f32, tag="ps")
            nc.tensor.matmul(prime_ps[0:32, 0, 0:288],
                             wall_t[:, 0:32], wall_t[:, 0:288],
                             start=True, stop=True, tile_position=(0, 0))
            prime_o = op.tile([128, NRND, 512], f16, tag="out")
            nc.vector.tensor_scalar_add(prime_o[:, 0, 0:1], b1_t[:], 0.0)
            nc.scalar.add(prime_o[:, 1, 0:1], b1_t[:], b1_t[:, 0:1])

            # Supertiles: up to 4 blocks per input tile/DMA.  Software
            # pipeline: DMA prefetch ~2 supertiles ahead; the on-chip dup
            # for st+1 is issued BEFORE the body of st so the strict-FIFO
            # DVE queue never head-of-line blocks on an input DMA.
            SUPS = [(4 * g, 4) for g in range(NBLK // 4)]
            if NBLK % 4:
                SUPS.append((NBLK - NBLK % 4, NBLK % 4))
            tiles = {}

            def dma_in(s):
                g0, nb = SUPS[s]
                tiles[s] = ip.tile([2 * KP, nb * TW], f16, tag="in",
                                   name=f"it{s}")
                # two 32-partition writes -> all 16 SDMA engines; on
                # separate HWDGE rings (sync/scalar) so they overlap
                # (HWDGE executes FIFO per issuing engine)
                nc.sync.dma_start(tiles[s][0:32, :],
                                  x_d[0:32, g0 * TW:(g0 + nb) * TW])
                nc.scalar.dma_start(tiles[s][64:96, :],
                                    x_d[32:64, g0 * TW:(g0 + nb) * TW])

            def dup(s, nchunk=1):
                # copy1[p, c] = copy0[p, c+1] per quarter; all on DVE
                # (fp16 copies: DVE ~4.7 col/ns vs ACT ~1.1 col/ns, and
                # ACT is already loaded with 10 of 12 evictions).  For the
                # first supertiles, split into column chunks so the first
                # matmuls unlock after ~1us instead of ~5us.
                g0, nb = SUPS[s]
                it = tiles[s]
                wtot = nb * TW
                edges = [wtot * k // nchunk for k in range(nchunk)] + [wtot - 1]
                for a, bb in zip(edges, edges[1:]):
                    nc.vector.tensor_scalar_add(
                        it[32:64, a:bb], it[0:32, a + 1:bb + 1], 0.0)
                    nc.vector.tensor_scalar_add(
                        it[96:128, a:bb], it[64:96, a + 1:bb + 1], 0.0)

            dma_in(0)
            dup(0, nchunk=8)
            dma_in(1)
            dup(1, nchunk=4)
            for st in range(len(SUPS)):
                if st + 2 < len(SUPS):
                    dma_in(st + 2)
                    dup(st + 2, nchunk=4)
                g0, nb = SUPS[st]
                it = tiles.pop(st)
                for half in range(nb):
                    cbase = half * TW
                    sup, hh = divmod(g0 + half, 2)
                    if hh == 0:
                        ot = op.tile([128, 2 * NRND, 512], f16, tag="out",
                                     name=f"ot{g0}_{half}")
                    for rp2 in range(NRND // 2):  # round pairs -> 2 banks
                        ps = pp.tile([128, 2, 512], f32, tag="ps")
                        for sub in range(2):
                            rnd = 2 * rp2 + sub
                            # 4 tasks: t = 4*rnd+cg; task t = (pair, rpair)
                            for kx in range(3):   # kx offsets d = 0, 2, 4
                                for cg in range(4):
                                    t = 4 * rnd + cg
                                    pair, rp = divmod(t, 3)
                                    b = cbase + 512 * pair + 2 * kx
                                    nc.tensor.matmul(
                                        ps[32 * cg:32 * cg + 32, sub, :],
                                        wall_t[:, (3 * kx + rp) * 32:
                                               (3 * kx + rp) * 32 + 32],
                                        it[:, b:b + 512],
                                        start=(kx == 0), stop=(kx == 2),
                                        tile_position=(0, 32 * cg))
                        # evict both banks with ONE instruction (halves
                        # per-instr overhead; halo cols discarded on
                        # host), bias fused, fp16 cast.  DVE carries the
                        # dup copies, so it gets 1 of 6 pairs; ACT 5.
                        dst = ot[:, NRND * hh + 2 * rp2:
                                 NRND * hh + 2 * rp2 + 2, :]
                        if hh == 0 and rp2 == 0:
                            # DVE pair as two single-bank ops (DVE's
                            # 2-bank read is anomalously slow)
                            nc.vector.tensor_scalar_add(
                                dst[:, 0, :], ps[:, 0, :], b1_t[:, 0:1])
                            nc.vector.tensor_scalar_add(
                                dst[:, 1, :], ps[:, 1, :], b1_t[:, 0:1])
                        else:
                            nc.scalar.add(dst, ps[:], b1_t[:, 0:1])
                    # per-block output DMA, alternating HWDGE rings so
                    # consecutive blocks overlap and rings stay balanced
                    if (g0 + half) % 2 == 0:
                        nc.sync.dma_start(o_d[sup][:, 0:NRND, :],
                                          ot[:, 0:NRND, :])
                    else:
                        nc.scalar.dma_start(o_d[sup][:, NRND:2 * NRND, :],
                                            ot[:, NRND:2 * NRND, :])
    nc.compile()
    return nc


def _get_module():
    global _STATE
    if _STATE is None:
        _STATE = _build_module()
    return _STATE


def kernel(x, w3, b3, w4, b4, w6, b6):
    from concourse.bass_utils import run_bass_kernel_spmd

    x = np.asarray(x, np.float32)
    kd = _dense_kernel(np.asarray(w3, np.float32), np.asarray(w4, np.float32),
                       np.asarray(w6, np.float32))
    bias = np.concatenate([np.asarray(b3, np.float32),
                           np.asarray(b4, np.float32),
                           np.asarray(b6, np.float32)])

    wall = np.concatenate(
        [_wall(kd, d, rp) for d in (0, 2, 4) for rp in range(3)],
        axis=1).astype(np.float16)
    # psum partition p = 32*cg + co*2 + rloc -> bias[co]
    b1 = bias[(np.arange(128) % 32) // 2].astype(np.float32).reshape(128, 1)

    nc = _get_module()
    x16 = x.astype(np.float16)
    in_maps = []
    for cr in range(NCORES):
        xs = x16[cr * BPC:(cr + 1) * BPC]
        # rows[(h, c), j*256 + w] = x[j, c, h, w]
        rows = np.ascontiguousarray(
            xs.transpose(2, 1, 0, 3)).reshape(H * C, BPC * W)
        xstk = np.zeros((KP, NBLK, TW), np.float16)
        for g in range(NBLK):
            blk = rows[R * C * g: R * C * g + KK]
            xstk[0:KK, g, 4:4 + BPC * W] = blk
        in_maps.append({"x": xstk.reshape(KP, NBLK * TW),
                        "wall": wall, "b1": b1})
    res = run_bass_kernel_spmd(nc, in_maps, core_ids=list(range(NCORES)))
    global LAST_RESULT
    LAST_RESULT = res

    # Unpack: o[sup, half, rnd, p, j*252+w]:
    #   task t = 4*rnd + p//32; pair = t//3; rp = t%3; img = 2*pair + j
    #   co = (p%32)//2; r = 6*(2*sup+half) + 2*rp + (p%2)
    out = np.empty((B, CO, HO, WO), np.float32)
    t_idx = np.arange(NRND * 128) // 32        # task for (rnd, p)
    p_idx = np.arange(NRND * 128) % 128
    pair = t_idx // 3
    rp = t_idx % 3
    co = (p_idx % 32) // 2
    rloc = p_idx % 2
    rr = 2 * rp + rloc                          # row within block (0..5)
    for cr in range(NCORES):
        o = res.results[cr]["o"].astype(np.float32)   # [NSUP, 128, 12, 512]
        o = o.reshape(NSUP, 128, 2, NRND, 2, 256)[..., 4:256]
        o = o.transpose(0, 2, 3, 1, 4, 5).reshape(NBLK, NRND * 128, 2, 252)
        img = (2 * pair[None, :, None] + np.arange(2)[None, None, :])
        blk = np.arange(NBLK)[:, None, None]
        out[cr * BPC + img, co[None, :, None], 6 * blk + rr[None, :, None]] = o
    return out


LAST_RESULT = None



# revision 3
# speedup vs baseline: 1.3407x; 1.3407x over previous
"""Trainium2 Bass kernel for the LeNet C3 dense-conv layer.

Computes out = conv2d_valid(x, K, stride 1) + bias where K is the dense
[16, 6, 5, 5] kernel scattered from the sparse per-branch weights
(w3/w4/w6), x is [128, 6, 256, 256] f32, out is [128, 16, 252, 252] f32.

Strategy (v2):
  - Pure data parallelism: 16 images per NeuronCore across 8 cores.
  - Conv as shift-accumulated banded matmuls into PSUM, as in v1, but
    with COLUMN-GROUP TILED matmuls: instead of one M=96 matmul per
    image-pair x kx-pair, issue four concurrent M=32 matmuls (one per
    32-column PE array group, tile_position=(0,32c)) covering four
    (image-pair, r-pair) tasks at once into the four 32-partition
    slices of one PSUM bank.  Measured on HW: per-pipe issue cadence
    ~259 ns for N=512 with 4 pipes overlapped -> ~742 ns per 12-MM
    round vs ~3x287 ns for the M=96 serial form (1.55x tensor-engine
    speedup; LDWEIGHTS is not hidden in the serial form).
  - K=120 = two stacked copies of the 10 input rows (60 partitions
    each), second copy pre-shifted one column, so each matmul covers
    two kernel columns kx: 3 matmuls per task (kx {0,1} {2,3} {4}).
    K=120 > 96 keeps all four PE row-group quarters streaming at full
    rate (K<=96 measures at half rate on HW).
  - Input is staged to DRAM ONCE (60 rows, no duplication) and the
    shifted second copy is built ON-CHIP by a GpSimd copy (SBUF->SBUF,
    does not touch HBM/DMA).  This halves input HBM traffic vs v1:
    total DMA drops 73.9 MB -> 53.2 MB per core (DMA was 96% busy).
  - Two row-blocks (12 output rows) share one input tile / one input
    DMA (~1 MB) and one output DMA (~1.5 MB) for DMA efficiency.
  - PSUM bank [128, 512] per round; evictions alternate between the
    vector and scalar engines (different banks -> legal concurrent
    PSUM reads), bias add fused, fp16 cast.
  - fp16 operands (~3e-4 rel err; accumulation is fp32 in PSUM).
"""

import numpy as np

# LeNet-5 C3 sparse channel connectivity (from the model definition).
CH3 = np.array([[0, 1, 2], [1, 2, 3], [2, 3, 4], [3, 4, 5], [0, 4, 5], [0, 1, 5]])
CH4 = np.array([[0, 1, 2, 3], [1, 2, 3, 4], [2, 3, 4, 5], [0, 3, 4, 5],
                [0, 1, 4, 5], [0, 1, 2, 5], [0, 1, 3, 4], [1, 2, 4, 5],
                [0, 2, 3, 5]])

B, C, H, W = 128, 6, 256, 256
CO, HO, WO = 16, 252, 252
NCORES = 8
BPC = B // NCORES           # images per core (16)
KH = KW = 5

R = 6                       # output rows per block
HI = R + 4                  # input rows per block (10)
NBLK = HO // R              # 42 blocks
NSUP = NBLK // 2            # 21 superblocks (2 blocks each)
KK = C * HI                 # contraction rows per kx copy (60)
KP = 64                     # copy-0 rows padded to 64 (32-aligned engine APs)
TW = 4 + BPC * W + 1        # input tile width per block (4101, last col zero)
NRND = 6                    # PSUM rounds per block (4 tasks each)

_STATE = None  # cached Bass module so repeat kernel() calls skip re-tracing


def _dense_kernel(w3, w4, w6):
    k = np.zeros((CO, C, KH, KW), np.float32)
    k[np.arange(6)[:, None], CH3] = w3
    k[6 + np.arange(9)[:, None], CH4] = w4
    k[15] = w6[0]
    return k


# Tile partition layout: quarters [0:32]=copy0 rows 0..31,
# [32:64]=copy1 rows 0..31, [64:96]=copy0 rows 32..63,
# [96:128]=copy1 rows 32..63.  Input DMA writes quarters 0 and 2
# (partition ranges 0..31 and 64..95 -> all 16 SDMA engines; a plain
# [0:64] write would hit only the 8 even engines).  The dup engine
# copies build quarters 1 and 3.  Row index = i*6 + ci (>=KK is pad).


def _wall(kd, d, rp):
    """Banded lhsT [2*KP, 32] for kx offset d and r-pair rp, with the
    quartered partition layout above.  Column co*2 + rloc, value
    kd[co, ci, i - (2*rp + rloc), d + s]."""
    out = np.zeros((2 * KP, 32), np.float32)
    for p in range(2 * KP):
        q, r = divmod(p, 32)
        s = q % 2                   # copy (kx shift)
        row = 32 * (q // 2) + r     # i*6 + ci
        if row >= KK:
            continue
        i, ci = divmod(row, C)
        kx = d + s
        if kx >= KW:
            continue
        for rloc in range(2):
            ky = i - (2 * rp + rloc)
            if 0 <= ky < KH:
                out[p, np.arange(CO) * 2 + rloc] = kd[:, ci, ky, kx]
    return out


def _build_module():
    import concourse.bacc as bacc
    import concourse.mybir as mybir
    from concourse.tile import TileContext

    f32 = mybir.dt.float32
    f16 = mybir.dt.float16

    nc = bacc.Bacc(None)
    # Single-copy per-block input rows, flat: x[r, g*TW + c] = block g
    # row r col c.  Supertiles of up to 4 blocks share one input DMA.
    x_d = nc.dram_tensor("x", [KP, NBLK * TW], f16, kind="ExternalInput")
    # walls: [(d, rpair) -> [120, 32]] flattened to [120, 9*32]
    wall_d = nc.dram_tensor("wall", [2 * KP, 9 * 32], f16, kind="ExternalInput")
    b1_d = nc.dram_tensor("b1", [128, 1], f32, kind="ExternalInput")
    # o[sup, p, half*NRND+rnd, j*256+w'] fp16 (full PSUM banks incl. halo
    # cols; host slices w' 4:256).  Host unpacks (see kernel()).
    o_d = nc.dram_tensor("o", [NSUP, 128, 2 * NRND, 512], f16,
                         kind="ExternalOutput")

    with TileContext(nc) as tc:
        with (
            tc.tile_pool(name="wpool", bufs=1) as wp,
            tc.tile_pool(name="inpool", bufs=4) as ip,
            tc.tile_pool(name="outpool", bufs=3) as op,
            tc.tile_pool(name="pspool", bufs=4, space="PSUM") as pp,
        ):
            wall_t = wp.tile([2 * KP, 9 * 32], f16)
            nc.sync.dma_start(wall_t[:], wall_d[:])
            b1_t = wp.tile([128, 1], f32)
            nc.sync.dma_start(b1_t[:], b1_d[:])

            # Prime the engines / constant tiles so steady-state
            # instructions carry few semaphore waits.
            prime_ps = pp.tile([128, 2, 512], f32, tag="ps")
            nc.tensor.matmul(prime_ps[0:32, 0, 0:288],
                             wall_t[:, 0:32], wall_t[:, 0:288],
                             start=True, stop=True, tile_position=(0, 0))
            prime_o = op.tile([128, NRND, 512], f16, tag="out")
            nc.vector.tensor_scalar_add(prime_o[:, 0, 0:1], b1_t[:], 0.0)
            nc.scalar.add(prime_o[:, 1, 0:1], b1_t[:], b1_t[:, 0:1])

            # Supertiles: up to 4 blocks per input tile/DMA.  Software
            # pipeline: DMA prefetch ~2 supertiles ahead; the on-chip dup
            # for st+1 is issued BEFORE the body of st so the strict-FIFO
            # DVE queue never head-of-line blocks on an input DMA.
            SUPS = [(4 * g, 4) for g in range(NBLK // 4)]
            if NBLK % 4:
                SUPS.append((NBLK - NBLK % 4, NBLK % 4))
            tiles = {}

            def dma_in(s):
                g0, nb = SUPS[s]
                tiles[s] = ip.tile([2 * KP, nb * TW], f16, tag="in",
                                   name=f"it{s}")
                # two 32-partition writes -> all 16 SDMA engines; both on
                # the otherwise-idle GpSimd ring so input triggers never
                # share a queue with output triggers or compute (a WAR
                # wait here must not head-of-line block evictions)
                nc.gpsimd.dma_start(tiles[s][0:32, :],
                                    x_d[0:32, g0 * TW:(g0 + nb) * TW])
                nc.gpsimd.dma_start(tiles[s][64:96, :],
                                    x_d[32:64, g0 * TW:(g0 + nb) * TW])

            def dup(s, nchunk=1):
                # copy1[p, c] = copy0[p, c+1] per quarter; all on DVE
                # (fp16 copies: DVE ~4.7 col/ns vs ACT ~1.1 col/ns, and
                # ACT is already loaded with 10 of 12 evictions).  For the
                # first supertiles, split into column chunks so the first
                # matmuls unlock after ~1us instead of ~5us.
                g0, nb = SUPS[s]
                it = tiles[s]
                wtot = nb * TW
                edges = [wtot * k // nchunk for k in range(nchunk)] + [wtot - 1]
                for a, bb in zip(edges, edges[1:]):
                    nc.vector.tensor_scalar_add(
                        it[32:64, a:bb], it[0:32, a + 1:bb + 1], 0.0)
                    nc.vector.tensor_scalar_add(
                        it[96:128, a:bb], it[64:96, a + 1:bb + 1], 0.0)

            dma_in(0)
            dup(0, nchunk=8)
            dma_in(1)
            dup(1, nchunk=4)
            for st in range(len(SUPS)):
                if st + 2 < len(SUPS):
                    dma_in(st + 2)
                    dup(st + 2, nchunk=4)
                g0, nb = SUPS[st]
                it = tiles.pop(st)
                for half in range(nb):
                    cbase = half * TW
                    sup, hh = divmod(g0 + half, 2)
                    if hh == 0:
                        ot = op.tile([128, 2 * NRND, 512], f16, tag="out",
                                     name=f"ot{g0}_{half}")
                    for rp2 in range(NRND // 2):  # round pairs -> 2 banks
                        ps = pp.tile([128, 2, 512], f32, tag="ps")
                        for sub in range(2):
                            rnd = 2 * rp2 + sub
                            # 4 tasks: t = 4*rnd+cg; task t = (pair, rpair)
                            for kx in range(3):   # kx offsets d = 0, 2, 4
                                for cg in range(4):
                                    t = 4 * rnd + cg
                                    pair, rp = divmod(t, 3)
                                    b = cbase + 512 * pair + 2 * kx
                                    nc.tensor.matmul(
                                        ps[32 * cg:32 * cg + 32, sub, :],
                                        wall_t[:, (3 * kx + rp) * 32:
                                               (3 * kx + rp) * 32 + 32],
                                        it[:, b:b + 512],
                                        start=(kx == 0), stop=(kx == 2),
                                        tile_position=(0, 32 * cg))
                        # evict both banks with ONE instruction (halves
                        # per-instr overhead; halo cols discarded on
                        # host), bias fused, fp16 cast.  All on ACT so
                        # the DVE queue carries only dup copies and can
                        # never stall evictions behind an input DMA wait.
                        dst = ot[:, NRND * hh + 2 * rp2:
                                 NRND * hh + 2 * rp2 + 2, :]
                        nc.scalar.add(dst, ps[:], b1_t[:, 0:1])
                    # per-block output DMA, both halves on the sync ring
                    # (which is otherwise idle after startup); ACT's queue
                    # carries no DMA triggers at all
                    if (g0 + half) % 2 == 0:
                        nc.sync.dma_start(o_d[sup][:, 0:NRND, :],
                                          ot[:, 0:NRND, :])
                    else:
                        nc.sync.dma_start(o_d[sup][:, NRND:2 * NRND, :],
                                          ot[:, NRND:2 * NRND, :])
    nc.compile()
    return nc


def _get_module():
    global _STATE
    if _STATE is None:
        _STATE = _build_module()
    return _STATE


def kernel(x, w3, b3, w4, b4, w6, b6):
    from concourse.bass_utils import run_bass_kernel_spmd

    x = np.asarray(x, np.float32)
    kd = _dense_kernel(np.asarray(w3, np.float32), np.asarray(w4, np.float32),
                       np.asarray(w6, np.float32))
    bias = np.concatenate([np.asarray(b3, np.float32),
                           np.asarray(b4, np.float32),
                           np.asarray(b6, np.float32)])

    wall = np.concatenate(
        [_wall(kd, d, rp) for d in (0, 2, 4) for rp in range(3)],
        axis=1).astype(np.float16)
    # psum partition p = 32*cg + co*2 + rloc -> bias[co]
    b1 = bias[(np.arange(128) % 32) // 2].astype(np.float32).reshape(128, 1)

    nc = _get_module()
    x16 = x.astype(np.float16)
    in_maps = []
    for cr in range(NCORES):
        xs = x16[cr * BPC:(cr + 1) * BPC]
        # rows[(h, c), j*256 + w] = x[j, c, h, w]
        rows = np.ascontiguousarray(
            xs.transpose(2, 1, 0, 3)).reshape(H * C, BPC * W)
        xstk = np.zeros((KP, NBLK, TW), np.float16)
        for g in range(NBLK):
            blk = rows[R * C * g: R * C * g + KK]
            xstk[0:KK, g, 4:4 + BPC * W] = blk
        in_maps.append({"x": xstk.reshape(KP, NBLK * TW),
                        "wall": wall, "b1": b1})
    res = run_bass_kernel_spmd(nc, in_maps, core_ids=list(range(NCORES)))
    global LAST_RESULT
    LAST_RESULT = res

    # Unpack: o[sup, half, rnd, p, j*252+w]:
    #   task t = 4*rnd + p//32; pair = t//3; rp = t%3; img = 2*pair + j
    #   co = (p%32)//2; r = 6*(2*sup+half) + 2*rp + (p%2)
    out = np.empty((B, CO, HO, WO), np.float32)
    t_idx = np.arange(NRND * 128) // 32        # task for (rnd, p)
    p_idx = np.arange(NRND * 128) % 128
    pair = t_idx // 3
    rp = t_idx % 3
    co = (p_idx % 32) // 2
    rloc = p_idx % 2
    rr = 2 * rp + rloc                          # row within block (0..5)
    for cr in range(NCORES):
        o = res.results[cr]["o"].astype(np.float32)   # [NSUP, 128, 12, 512]
        o = o.reshape(NSUP, 128, 2, NRND, 2, 256)[..., 4:256]
        o = o.transpose(0, 2, 3, 1, 4, 5).reshape(NBLK, NRND * 128, 2, 252)
        img = (2 * pair[None, :, None] + np.arange(2)[None, None, :])
        blk = np.arange(NBLK)[:, None, None]
        out[cr * BPC + img, co[None, :, None], 6 * blk + rr[None, :, None]] = o
    return out


LAST_RESULT = None

